# revision 15
# baseline (speedup 1.0000x reference)
"""Trainium2 Bass kernel for nn_CustomConv: 3x3 same-padding conv.

Full problem: input [32, 32, 128, 128] f32, weight [64, 32, 3, 3] f32
-> output [32, 64, 128, 128] f32.

Sharding: data-parallel across 8 NeuronCores on the batch axis (4 images
per core); the small weight tensor is replicated.

Per-core kernel design:
  * The conv is computed as 3 PSUM-accumulating matmuls per output tile,
    contracting over (dx, ci) = 3*32 = 96 partitions. The dy taps become
    plain row offsets into a row-padded SBUF image buffer, so the rhs of
    each matmul is a contiguous slice.
  * SBUF image buffer layout (per image, fp16): partitions p = dx*32+ci,
    each holding (H+2) x W values: buf[p][r, x] = in[ci, r-1, x+dx-1]
    (zero-padded outside the image). The dx=1 (center) group is loaded
    from HBM with a casting DMA (f32 -> f16); dx=0/dx=2 groups are
    on-chip shifted copies (SBUF->SBUF DMA) plus small edge memsets.
  * Output tile = [128, 512] PSUM: col-groups 0-1 hold rows 4r..4r+3
    (64 output channels), col-groups 2-3 hold rows 4r+4..4r+7. The two
    64-wide matmuls per dy run on different PE column groups and overlap.
  * PSUM -> SBUF evacuation alternates Vector/Scalar engines; two tiles
    are batched per 512 KiB output DMA.
"""

import numpy as np

import concourse.bass as bass
import concourse.mybir as mybir
from concourse.tile import TileContext

F32 = mybir.dt.float32
F16 = mybir.dt.float16

B, CIN, H, W = 32, 32, 128, 128
COUT, KS = 64, 3
NCORES = 8
BPC = B // NCORES  # images per core

_CACHE = {}


def build_nc(bpc=BPC, h=H, split_waits=True):
    """Build the per-core Bass module. bpc/h are parameterized only for
    small-scale simulation tests; hardware uses the defaults.
    split_waits rewrites multi-wait instructions for walrus encoding
    limits (CoreSim can't execute the NoOp form, so sim tests disable)."""
    hp = h + 2  # padded rows
    nc = bass.Bass()
    x = nc.declare_dram_parameter("x", [bpc, CIN, h, W], F32, isOutput=False)
    wts = nc.declare_dram_parameter("w", [96, 384], F16, isOutput=False)
    y = nc.declare_dram_parameter("y", [bpc, COUT, h, W], F32, isOutput=True)

    x_flat = x.ap().rearrange("b c h w -> b c (h w)")
    y_flat = y.ap().rearrange("b c h w -> b c (h w)")

    n_pairs = h // 8  # each psum tile covers 8 output rows

    with TileContext(nc) as tc:
        with (
            tc.tile_pool(name="wpool", bufs=1) as wpool,
            tc.tile_pool(name="inpool", bufs=2) as inpool,
            tc.tile_pool(name="stpool", bufs=3) as stpool,
            tc.tile_pool(name="psum", bufs=4, space="PSUM") as psum_pool,
            tc.tile_pool(name="scr", bufs=1, space="PSUM") as scr_pool,
        ):
            wt = wpool.tile([96, 384], F16)
            nc.sync.dma_start(out=wt, in_=wts.ap())

            # The MM instruction encoding has very few sync-wait slots, and
            # the first matmul of each image would otherwise wait on every
            # producer of the image buffer (3 DMA sem lanes + DVE memsets).
            # These 1-elem dummy matmuls sit earlier in the PE queue and
            # absorb one or two producer sems each, so every real matmul
            # carries <=2 waits.
            scr = scr_pool.tile([1, 16], F32)

            def absorb(j, src):
                nc.tensor.matmul(
                    scr[0:1, j : j + 1],
                    lhsT=src,
                    rhs=src,
                    start=True,
                    stop=True,
                    skip_group_check=True,
                )

            for b in range(bpc):
                buf = inpool.tile([96, hp * W], F16, tag="img")
                body = h * W  # image body element count
                # 1) center (dx=1) load: rows 1..h, casting f32->f16
                nc.gpsimd.dma_start(
                    out=buf[32:64, W : W + body], in_=x_flat[b]
                )
                # 2) zero center pad row h+1 (read by the dx=2 shift below)
                nc.vector.memset(buf[32:64, (hp - 1) * W : hp * W], 0.0)
                # 3) dx=0 replica: buf0[f] = center[f-1]
                nc.sync.dma_start(
                    out=buf[0:32, W + 1 : W + 1 + body],
                    in_=buf[32:64, W : W + body],
                )
                # 4) dx=2 replica: buf2[f] = center[f+1] (last src elem is
                #    the zeroed pad row)
                nc.sync.dma_start(
                    out=buf[64:96, W : W + body],
                    in_=buf[32:64, W + 1 : W + 1 + body],
                )
                # 5) edge fixups
                # M1: row 0 for all groups
                nc.vector.memset(buf[0:96, 0:W], 0.0)
                # M2: row h+1 for all groups (dx=0 copy spilled one elem
                # into it; dx groups otherwise stale)
                nc.vector.memset(buf[0:96, (hp - 1) * W : hp * W], 0.0)
                # M3: column x=0 of dx=0 group, rows 1..h
                col0 = buf[0:32, W : (hp - 1) * W].rearrange(
                    "p (r x) -> p r x", x=W
                )[:, :, 0:1]
                nc.vector.memset(col0, 0.0)
                # M4: column x=W-1 of dx=2 group, rows 1..h
                colw = buf[64:96, 2 * W - 1 : hp * W - 1].rearrange(
                    "p (r x) -> p r x", x=W
                )[:, :, 0:1]
                nc.vector.memset(colw, 0.0)

                # sem absorbers (see above): one per producer proc/region
                absorb(0, wt[0:96, 0:1])              # weight DMA
                absorb(1, buf[32:64, W : W + 1])      # center load (SWDGE)
                absorb(2, buf[0:32, W + 1 : W + 2])   # dx=0 replica DMA
                absorb(3, buf[0:96, 0:1])             # M1
                absorb(4, buf[0:96, (hp - 1) * W + 64 : (hp - 1) * W + 65])  # M2
                absorb(5, buf[0:32, W : W + 1])       # M3
                absorb(6, buf[64:96, 2 * W - 1 : 2 * W])  # M4 + dx=2 DMA

                # compute: 8 output rows per psum tile
                for pp in range(n_pairs // 2):
                    st = stpool.tile([128, 1024], F32, tag="st")
                    for q in range(2):
                        p = 2 * pp + q  # pair index: output rows 8p..8p+7
                        ps = psum_pool.tile([128, 512], F32, tag="ps")
                        for dy in range(3):
                            # block A: rows 8p..8p+3; block B: rows 8p+4..+7
                            rA = (8 * p + dy) * W
                            rB = (8 * p + 4 + dy) * W
                            nc.tensor.matmul(
                                ps[0:64, :],
                                lhsT=wt[:, dy * 128 : dy * 128 + 64],
                                rhs=buf[0:96, rA : rA + 512],
                                start=(dy == 0),
                                stop=(dy == 2),
                                skip_group_check=True,
                            )
                            nc.tensor.matmul(
                                ps[64:128, :],
                                lhsT=wt[:, dy * 128 + 64 : dy * 128 + 128],
                                rhs=buf[0:96, rB : rB + 512],
                                start=(dy == 0),
                                stop=(dy == 2),
                                skip_group_check=True,
                            )
                        # evacuate PSUM; alternate engines
                        dst = st[:, q * 512 : q * 512 + 512]
                        if q == 0:
                            nc.vector.tensor_copy(out=dst, in_=ps)
                        else:
                            nc.scalar.copy(dst, ps)
                    # store 16 output rows: dest (k c) (q e) with k=block,
                    # c=cout, q=pair-in-tile, e=512
                    dst4 = y_flat[b][
                        :, 2048 * pp : 2048 * pp + 2048
                    ].rearrange("c (q k e) -> c q k e", q=2, k=2)
                    for k in range(2):
                        nc.sync.dma_start(
                            out=dst4[:, :, k, :], in_=st[64 * k : 64 * k + 64, :]
                        )
    if split_waits:
        _split_waits(nc)
    return nc


# Per-instruction-struct HW sync-wait slot limits are small (walrus
# "Too many sync wait commands"). Split excess waits onto standalone
# NoOp instructions queued just before, on the same engine.
_WAIT_LIMIT = {}
_SKIP_SPLIT = {
    "InstEventSemaphore",
    "InstAllEngineBarrier",
    "InstUnconditionalBranch",
    "InstNoOp",
}


def _split_waits(nc):
    n = 0
    for f in nc.m.functions:
        for blk in f.blocks:
            new = []
            for inst in blk.instructions:
                si = getattr(inst, "sync_info", None)
                tname = type(inst).__name__
                if si is not None and si.on_wait and tname not in _SKIP_SPLIT:
                    limit = _WAIT_LIMIT.get(tname, 1)
                    if len(si.on_wait) > limit:
                        extra, keep = si.on_wait[:-limit], si.on_wait[-limit:]
                        for w in extra:
                            n += 1
                            new.append(
                                mybir.InstNoOp(
                                    name=f"wsplit-{n}",
                                    engine=inst.engine,
                                    sync_info=mybir.SyncInfo(
                                        on_wait=[w], on_update=[]
                                    ),
                                    bass_nofuse=True,
                                )
                            )
                        inst.sync_info = mybir.SyncInfo(
                            on_wait=keep, on_update=si.on_update
                        )
                new.append(inst)
            blk.instructions[:] = new
    return n


def _prep_weights(kernel):
    # wts[dx*32+ci, dy*128 + j*64 + co] = kernel[co, ci, dy, dx], j in {0,1}
    w = kernel.astype(np.float16)
    arr = np.transpose(w, (3, 1, 2, 0)).reshape(96, 3, 64)  # [dx*ci, dy, co]
    return np.ascontiguousarray(np.tile(arr, (1, 1, 2)).reshape(96, 384))


def run(input, kernel, **spmd_kwargs):
    """Run the kernel on 8 NeuronCores; returns (output, BassKernelResults)."""
    from concourse.bass_utils import run_bass_kernel_spmd

    if "nc" not in _CACHE:
        _CACHE["nc"] = build_nc()
    nc = _CACHE["nc"]

    inp = np.ascontiguousarray(input.reshape(NCORES, BPC, CIN, H, W))
    wts = _prep_weights(kernel)
    in_maps = [{"x": inp[c], "w": wts} for c in range(NCORES)]
    bkr = run_bass_kernel_spmd(nc, in_maps, list(range(NCORES)), **spmd_kwargs)
    out = np.concatenate([bkr.results[c]["y"] for c in range(NCORES)], axis=0)
    return out.reshape(B, COUT, H, W), bkr


def kernel(input, kernel):
    return run(input, kernel)[0]


# revision 17
# speedup vs baseline: 1.0176x; 1.0176x over previous
"""Trainium2 Bass kernel for nn_CustomConv: 3x3 same-padding conv.

Full problem: input [32, 32, 128, 128] f32, weight [64, 32, 3, 3] f32
-> output [32, 64, 128, 128] f32.

Sharding: data-parallel across 8 NeuronCores on the batch axis (4 images
per core); the small weight tensor is replicated.

Per-core kernel design:
  * The conv is computed as 3 PSUM-accumulating matmuls per output tile,
    contracting over (dx, ci) = 3*32 = 96 partitions. The dy taps become
    plain row offsets into a row-padded SBUF image buffer, so the rhs of
    each matmul is a contiguous slice.
  * SBUF image buffer layout (per image, fp16): partitions p = dx*32+ci,
    each holding (H+2) x W values: buf[p][r, x] = in[ci, r-1, x+dx-1]
    (zero-padded outside the image). The dx=1 (center) group is loaded
    from HBM with a casting DMA (f32 -> f16); dx=0/dx=2 groups are
    on-chip shifted copies (SBUF->SBUF DMA) plus small edge memsets.
  * Output tile = [128, 512] PSUM: col-groups 0-1 hold rows 4r..4r+3
    (64 output channels), col-groups 2-3 hold rows 4r+4..4r+7. The two
    64-wide matmuls per dy run on different PE column groups and overlap.
  * PSUM -> SBUF evacuation alternates Vector/Scalar engines; two tiles
    are batched per 512 KiB output DMA.
"""

import numpy as np

import concourse.bass as bass
import concourse.mybir as mybir
from concourse.tile import TileContext

F32 = mybir.dt.float32
F16 = mybir.dt.float16

B, CIN, H, W = 32, 32, 128, 128
COUT, KS = 64, 3
NCORES = 8
BPC = B // NCORES  # images per core

_CACHE = {}


def build_nc(bpc=BPC, h=H, split_waits=True):
    """Build the per-core Bass module. bpc/h are parameterized only for
    small-scale simulation tests; hardware uses the defaults.
    split_waits rewrites multi-wait instructions for walrus encoding
    limits (CoreSim can't execute the NoOp form, so sim tests disable)."""
    assert h % 16 == 0
    hh = h // 2  # rows per half-image chain
    hp = hh + 2  # buffer rows incl halo
    sz = hp * W  # buffer elems per partition
    nc = bass.Bass()
    x = nc.declare_dram_parameter("x", [bpc, CIN, h, W], F32, isOutput=False)
    wts = nc.declare_dram_parameter("w", [96, 384], F16, isOutput=False)
    y = nc.declare_dram_parameter("y", [bpc, COUT, h, W], F32, isOutput=True)

    x_flat = x.ap().rearrange("b c h w -> b c (h w)")
    y_flat = y.ap().rearrange("b c h w -> b c (h w)")

    with TileContext(nc) as tc:
        with (
            tc.tile_pool(name="wpool", bufs=1) as wpool,
            tc.tile_pool(name="inpool", bufs=3) as inpool,
            tc.tile_pool(name="stpool", bufs=3) as stpool,
            tc.tile_pool(name="psum", bufs=4, space="PSUM") as psum_pool,
        ):
            wt = wpool.tile([96, 384], F16)
            nc.sync.dma_start(out=wt, in_=wts.ap())

            for b in range(bpc):
                for hf in range(2):
                    # buffer row r = image row hf*hh + r - 1 + hf; i.e. the
                    # chain covers output rows [hf*hh, hf*hh+hh) with one
                    # halo row on each side (zero at image edges).
                    r0c = 1 - hf  # dest start row of the HBM load
                    nrows = hh + 1  # rows loaded from HBM (one halo side)
                    src_r0 = max(hf * hh - 1, 0)
                    buf = inpool.tile([96, sz], F16, tag="img")
                    c_lo, c_hi = r0c * W, r0c * W + nrows * W
                    # center (dx=1) load, casting f32->f16
                    nc.gpsimd.dma_start(
                        out=buf[32:64, c_lo:c_hi],
                        in_=x_flat[b][:, src_r0 * W : (src_r0 + nrows) * W],
                    )
                    # dx=0 replica: buf0[f] = center[f-1]
                    d_lo, d_hi = c_lo + 1, min(c_hi + 1, sz)
                    nc.scalar.dma_start(
                        out=buf[0:32, d_lo:d_hi],
                        in_=buf[32:64, d_lo - 1 : d_hi - 1],
                    )
                    # dx=2 replica: buf2[f] = center[f+1]; src stays inside
                    # the loaded range, the dropped last dest elem is an
                    # x=W-1 edge the column memset below zeroes anyway
                    nc.scalar.dma_start(
                        out=buf[64:96, c_lo : c_hi - 1],
                        in_=buf[32:64, c_lo + 1 : c_hi],
                    )
                    # edge fixups (after copies; order matters for WAW)
                    # outer halo row (image top/bottom pad): zero
                    pr = (hp - 1) * W if hf else 0
                    nc.vector.memset(buf[0:96, pr : pr + W], 0.0)
                    # column x=0 of dx=0 group, all rows
                    col0 = buf[0:32, 0:sz].rearrange("p (r x) -> p r x", x=W)[
                        :, :, 0:1
                    ]
                    nc.vector.memset(col0, 0.0)
                    # column x=W-1 of dx=2 group, all rows
                    colw = buf[64:96, 0:sz].rearrange(
                        "p (r x) -> p r x", x=W
                    )[:, :, W - 1 : W]
                    nc.vector.memset(colw, 0.0)

                    # compute: 8 output rows per psum tile, 2 per store tile
                    for pp in range(hh // 16):
                        st = stpool.tile([128, 1024], F32, tag="st")
                        for q in range(2):
                            p = 2 * pp + q  # local pair: rows 8p..8p+7
                            ps = psum_pool.tile([128, 512], F32, tag="ps")
                            for dy in range(3):
                                rA = (8 * p + dy) * W
                                rB = (8 * p + 4 + dy) * W
                                nc.tensor.matmul(
                                    ps[0:64, :],
                                    lhsT=wt[:, dy * 128 : dy * 128 + 64],
                                    rhs=buf[0:96, rA : rA + 512],
                                    start=(dy == 0),
                                    stop=(dy == 2),
                                    skip_group_check=True,
                                )
                                nc.tensor.matmul(
                                    ps[64:128, :],
                                    lhsT=wt[:, dy * 128 + 64 : dy * 128 + 128],
                                    rhs=buf[0:96, rB : rB + 512],
                                    start=(dy == 0),
                                    stop=(dy == 2),
                                    skip_group_check=True,
                                )
                            # evacuate PSUM; alternate engines
                            dst = st[:, q * 512 : q * 512 + 512]
                            if q == 0:
                                nc.vector.tensor_copy(out=dst, in_=ps)
                            else:
                                nc.scalar.copy(dst, ps)
                        # store 16 output rows; spread across both HWDGE rings
                        col = (hf * hh + 16 * pp) * W
                        dst4 = y_flat[b][:, col : col + 2048].rearrange(
                            "c (q k e) -> c q k e", q=2, k=2
                        )
                        for k in range(2):
                            eng = nc.sync if (pp + k) % 2 == 0 else nc.scalar
                            eng.dma_start(
                                out=dst4[:, :, k, :],
                                in_=st[64 * k : 64 * k + 64, :],
                            )
    if split_waits:
        _split_waits(nc)
    return nc


# Per-instruction-struct HW sync-wait slot limits are small (walrus
# "Too many sync wait commands"). Split excess waits onto standalone
# NoOp instructions queued just before, on the same engine.
_WAIT_LIMIT = {}
_SKIP_SPLIT = {
    "InstEventSemaphore",
    "InstAllEngineBarrier",
    "InstUnconditionalBranch",
    "InstNoOp",
}


def _split_waits(nc):
    n = 0
    for f in nc.m.functions:
        for blk in f.blocks:
            new = []
            for inst in blk.instructions:
                si = getattr(inst, "sync_info", None)
                tname = type(inst).__name__
                if si is not None and si.on_wait and tname not in _SKIP_SPLIT:
                    limit = _WAIT_LIMIT.get(tname, 1)
                    if len(si.on_wait) > limit:
                        extra, keep = si.on_wait[:-limit], si.on_wait[-limit:]
                        for w in extra:
                            n += 1
                            new.append(
                                mybir.InstNoOp(
                                    name=f"wsplit-{n}",
                                    engine=inst.engine,
                                    sync_info=mybir.SyncInfo(
                                        on_wait=[w], on_update=[]
                                    ),
                                    bass_nofuse=True,
                                )
                            )
                        inst.sync_info = mybir.SyncInfo(
                            on_wait=keep, on_update=si.on_update
                        )
                new.append(inst)
            blk.instructions[:] = new
    return n


def _prep_weights(kernel):
    # wts[dx*32+ci, dy*128 + j*64 + co] = kernel[co, ci, dy, dx], j in {0,1}
    w = kernel.astype(np.float16)
    arr = np.transpose(w, (3, 1, 2, 0)).reshape(96, 3, 64)  # [dx*ci, dy, co]
    return np.ascontiguousarray(np.tile(arr, (1, 1, 2)).reshape(96, 384))


def run(input, kernel, **spmd_kwargs):
    """Run the kernel on 8 NeuronCores; returns (output, BassKernelResults)."""
    from concourse.bass_utils import run_bass_kernel_spmd

    if "nc" not in _CACHE:
        _CACHE["nc"] = build_nc()
    nc = _CACHE["nc"]

    inp = np.ascontiguousarray(input.reshape(NCORES, BPC, CIN, H, W))
    wts = _prep_weights(kernel)
    in_maps = [{"x": inp[c], "w": wts} for c in range(NCORES)]
    bkr = run_bass_kernel_spmd(nc, in_maps, list(range(NCORES)), **spmd_kwargs)
    out = np.concatenate([bkr.results[c]["y"] for c in range(NCORES)], axis=0)
    return out.reshape(B, COUT, H, W), bkr


def kernel(input, kernel):
    return run(input, kernel)[0]
